# revision 18
# baseline (speedup 1.0000x reference)
"""Multi-head self-attention Trainium2 kernel, 8-core SPMD.

Sharding: data-parallel over batch (2) x tensor-parallel over heads
(16 heads -> 4 per core).  Core c handles batch c//4, heads
[4*(c%4), 4*(c%4)+4).  Each core computes its 4 heads' attention and a
partial output projection; the host sums the 4 partials per batch and
adds the output bias.

Device layout (per core):
  xT   [1024, 2048]  x[batch].T (feature-major)
  wqkT [1024, 512]   Q,K weight cols as 4 m-tiles: Qh01 Qh23 Kh01 Kh23
  wvT  [1024, 256]   V weight cols (4 heads x 64)
  woT  [256, 1024]   out-proj rows for our 256 head-features
  bqkT [128, 4]      per-m-tile bias columns
  bv   [1, 256]      V bias row
  out  [2048, 1024]  partial output (pre-bias)

Attention math per (pair of heads, q-block of 512):
  S_T[k,q] = K[d,k].T @ Q[d,q] on PE (f32r), two heads packed in
  row-groups; exp via ScalarE with scale=1/8 (no max subtraction --
  scores are N(0,1)-ish, max ~6, exp safe); AV[d,q] accumulated over
  k-tiles with lhsT = [V_tok | ones] so PSUM row 64 is the softmax
  denominator for free; normalize with a broadcast reciprocal on DVE.
"""

import numpy as np

import concourse.bacc as bacc
import concourse.bass as bass
import concourse.mybir as mybir
import concourse.tile as tile
from concourse.bass_utils import run_bass_kernel_spmd

F32 = mybir.dt.float32
F32R = mybir.dt.float32r
BF16 = mybir.dt.bfloat16
EXP = mybir.ActivationFunctionType.Exp

import os
DT = BF16 if os.environ.get("KDT", "bf16") == "bf16" else F32R
import ml_dtypes
NP_DT = ml_dtypes.bfloat16 if DT is BF16 else np.float32

N_CORES = 8
S = 2048          # tokens per batch
D = 1024          # d_model
NKT = 16          # 128-token k tiles
NQB = 4           # 512-token q blocks
NKD = 8           # 128-feature contraction tiles of d_model

_CACHE = {}


def build(n_cores=N_CORES):
    nc = bacc.Bacc("TRN2", target_bir_lowering=False, num_devices=n_cores)
    xT = nc.declare_dram_parameter("xT", [D, S], DT, isOutput=False)
    wqkT = nc.declare_dram_parameter("wqkT", [D, 512], DT, isOutput=False)
    wvT = nc.declare_dram_parameter("wvT", [D, 256], DT, isOutput=False)
    woT = nc.declare_dram_parameter("woT", [256, D], DT, isOutput=False)
    bqkT = nc.declare_dram_parameter("bqkT", [128, 4], F32, isOutput=False)
    bv = nc.declare_dram_parameter("bv", [1, 256], DT, isOutput=False)
    onesA = nc.declare_dram_parameter("onesA", [1, 128], DT, isOutput=False)
    onesAr = nc.declare_dram_parameter("onesAr", [1, 64], F32R, isOutput=False)
    onesB = nc.declare_dram_parameter("onesB", [128, 4], DT, isOutput=False)
    out = nc.declare_dram_parameter("out", [S, D], F32, isOutput=True)

    with tile.TileContext(nc) as tc:
        with tc.tile_pool(name="const", bufs=1) as const, \
             tc.tile_pool(name="projp", bufs=2, space="PSUM") as projp, \
             tc.tile_pool(name="scp", bufs=2, space="PSUM") as scp, \
             tc.tile_pool(name="avp", bufs=2, space="PSUM") as avp, \
             tc.tile_pool(name="pp", bufs=4) as pp, \
             tc.tile_pool(name="dnp", bufs=2) as dnp, \
             tc.tile_pool(name="outsb", bufs=3) as outsb:

            xT_sb = [const.tile([128, S], DT, name=f"xt{k}", tag=f"xt{k}") for k in range(NKD)]
            wqk_sb = [const.tile([128, 512], DT, name=f"wqk{k}", tag=f"wqk{k}") for k in range(NKD)]
            wv_sb = [const.tile([128, 256], DT, name=f"wv{k}", tag=f"wv{k}") for k in range(NKD)]

            def dma_x_nb(nb):
                for k in range(NKD):
                    nc.sync.dma_start(
                        out=xT_sb[k][:, nb * 512:(nb + 1) * 512],
                        in_=xT.ap()[k * 128:(k + 1) * 128, nb * 512:(nb + 1) * 512],
                    )

            def dma_wqk_m(m):
                for k in range(NKD):
                    nc.sync.dma_start(
                        out=wqk_sb[k][:, m * 128:(m + 1) * 128],
                        in_=wqkT.ap()[k * 128:(k + 1) * 128, m * 128:(m + 1) * 128],
                    )

            dma_x_nb(0)
            dma_wqk_m(2)
            dma_wqk_m(0)
            for k in range(NKD):
                nc.sync.dma_start(out=wv_sb[k], in_=wvT.ap()[k * 128:(k + 1) * 128, :])
            dma_x_nb(1)
            dma_x_nb(2)
            dma_x_nb(3)
            dma_wqk_m(3)
            dma_wqk_m(1)
            bqk_sb = const.tile([128, 4], F32, name="bqk_sb")
            nc.sync.dma_start(out=bqk_sb, in_=bqkT.ap())
            ones_r = const.tile([1, 128], DT, name="ones_r")
            nc.sync.dma_start(out=ones_r, in_=onesA.ap())
            ones_rr = const.tile([1, 64], F32R, name="ones_rr")
            nc.sync.dma_start(out=ones_rr, in_=onesAr.ap())
            bv_sb = const.tile([1, 256], DT, name="bv_sb")
            nc.sync.dma_start(out=bv_sb, in_=bv.ap())
            wo_sb = []
            for p in range(2):
                wo = const.tile([128, D], DT, name=f"wo{p}")
                nc.sync.dma_start(out=wo, in_=woT.ap()[p * 128:(p + 1) * 128, :])
                wo_sb.append(wo)

            qk_sb = [const.tile([128, S], DT, name=f"qk{m}") for m in range(4)]
            # v tiles: [token-tile, head, 64 dims + ones col]
            v_sb = [const.tile([128, 4, 68], DT, name=f"v{t}") for t in range(NKT)]
            for t in range(NKT):
                nc.sync.dma_start(out=v_sb[t][:, :, 64:65], in_=onesB.ap())
            av_sb = [const.tile([128, S], DT, name=f"av{p}") for p in range(2)]

            def qk_group(m, nb):
                ps = projp.tile([128, 512], F32, tag="proj", name="ps_qk")
                for k in range(NKD):
                    nc.tensor.matmul(
                        ps,
                        wqk_sb[k][:, m * 128:(m + 1) * 128],
                        xT_sb[k][:, nb * 512:(nb + 1) * 512],
                        start=(k == 0),
                        stop=(k == NKD - 1),
                    )
                nc.vector.tensor_scalar_add(
                    out=qk_sb[m][:, nb * 512:(nb + 1) * 512],
                    in0=ps,
                    scalar1=bqk_sb[:, m:m + 1],
                )

            def v_group(t):
                ps = projp.tile([128, 512], F32, tag="proj", name="ps_v")
                psv = ps[:, 0:256]
                nc.tensor.matmul(psv, ones_r, bv_sb, start=True, stop=False)
                for k in range(NKD):
                    nc.tensor.matmul(
                        psv,
                        xT_sb[k][:, t * 128:(t + 1) * 128],
                        wv_sb[k],
                        start=False,
                        stop=(k == NKD - 1),
                    )
                nc.vector.tensor_copy(out=v_sb[t][:, :, 0:64], in_=ps[:, 0:256])

            def attn(pair, qb, extras=None, vjit_from=None):
                avs = [
                    avp.tile([65, 512], F32, tag="av", name="avA"),
                    avp.tile([65, 512], F32, tag="av", name="avB"),
                ]
                qs = qk_sb[pair]
                ks = qk_sb[2 + pair]
                extras = dict(extras or {})
                pts = []

                def av_mms(kt):
                    for h in (0, 1):
                        nc.tensor.matmul(
                            avs[h],
                            v_sb[kt][:, 2 * pair + h, 0:65],
                            pts[kt][:, h, :],
                            start=(kt == 0),
                            stop=(kt == NKT - 1),
                        )

                for kt in range(NKT):
                    sc = scp.tile([128, 2, 512], F32, tag="sc", name="sc")
                    for h in (0, 1):
                        nc.tensor.matmul(
                            sc[:, h, :],
                            ks[h * 64:(h + 1) * 64, kt * 128:(kt + 1) * 128],
                            qs[h * 64:(h + 1) * 64, qb * 512:(qb + 1) * 512],
                            start=True,
                            stop=True,
                        )
                    pt = pp.tile([128, 2, 512], DT, tag="p", name="pt")
                    nc.scalar.activation(out=pt, in_=sc, func=EXP, scale=0.125)
                    pts.append(pt)
                    if vjit_from is not None and kt >= vjit_from:
                        v_group(kt)
                    fn = extras.pop(kt, None)
                    if fn is not None:
                        fn()
                    if kt > 0:
                        av_mms(kt - 1)
                av_mms(NKT - 1)
                # Per head: copy the denominator row out of PSUM, broadcast it
                # to 64 partitions with a K=1 ones-matmul, take a fast approx
                # reciprocal (18 bits, plenty for a softmax denom), then scale.
                for h in (0, 1):
                    den = dnp.tile([1, 512], F32R, tag=f"den{h}", name="den")
                    nc.vector.tensor_copy(out=den, in_=avs[h][64:65, :])
                    rcp = projp.tile([64, 512], F32, tag="proj", name="rcp")
                    nc.tensor.matmul(rcp, ones_rr, den, start=True, stop=True)
                    rc = dnp.tile([64, 512], F32, tag=f"rc{h}", name="rc")
                    nc.vector.reciprocal_approx_fast(out=rc, in_=rcp)
                    nc.vector.tensor_mul(
                        out=av_sb[pair][h * 64:(h + 1) * 64, qb * 512:(qb + 1) * 512],
                        in0=avs[h][0:64, :],
                        in1=rc,
                    )

            def outproj_tt(qb, tt):
                ob_sb = outsb.tile([128, D], F32, tag="osb", name="ob_sb")
                tok = qb * 512 + tt * 128
                for ob in (0, 1):
                    ps = projp.tile([128, 512], F32, tag="proj", name="ps_o")
                    for p in (0, 1):
                        nc.tensor.matmul(
                            ps,
                            av_sb[p][:, tok:tok + 128],
                            wo_sb[p][:, ob * 512:(ob + 1) * 512],
                            start=(p == 0),
                            stop=(p == 1),
                        )
                    nc.vector.tensor_copy(out=ob_sb[:, ob * 512:(ob + 1) * 512], in_=ps)
                nc.sync.dma_start(out=out.ap()[tok:tok + 128, :], in_=ob_sb)

            def outproj_extras(qb):
                return [lambda tt=tt: outproj_tt(qb, tt) for tt in range(4)]

            qk_group(2, 0)               # K pair0 tiles kt 0-3
            qk_group(0, 0)               # Q pair0, qb0
            for t in range(8):
                v_group(t)
            attn(0, 0, extras={
                2: lambda: qk_group(2, 1), 5: lambda: qk_group(2, 2),
                8: lambda: qk_group(2, 3), 11: lambda: qk_group(3, 0),
                13: lambda: qk_group(1, 0),
            }, vjit_from=8)
            attn(1, 0, extras={
                2: lambda: qk_group(3, 1), 5: lambda: qk_group(3, 2),
                8: lambda: qk_group(3, 3), 12: lambda: qk_group(0, 1),
            })
            attn(0, 1, extras={3: lambda: qk_group(1, 1)})
            attn(1, 1, extras={
                3: lambda: qk_group(0, 2),
                6: lambda: outproj_tt(0, 0), 9: lambda: outproj_tt(0, 1),
                12: lambda: outproj_tt(0, 2), 14: lambda: outproj_tt(0, 3),
            })
            attn(0, 2, extras={3: lambda: qk_group(1, 2)})
            attn(1, 2, extras={
                3: lambda: qk_group(0, 3),
                6: lambda: outproj_tt(1, 0), 9: lambda: outproj_tt(1, 1),
                12: lambda: outproj_tt(1, 2), 14: lambda: outproj_tt(1, 3),
            })
            attn(0, 3, extras={3: lambda: qk_group(1, 3)})
            attn(1, 3, extras={
                6: lambda: outproj_tt(2, 0), 9: lambda: outproj_tt(2, 1),
                12: lambda: outproj_tt(2, 2), 14: lambda: outproj_tt(2, 3),
            })
            for tt in range(4):
                outproj_tt(3, tt)

    nc.compile()
    return nc


def make_in_maps(x, w_qkv, b_qkv, w_out):
    """Shard FULL inputs into per-core input dicts (host-side, free)."""
    x = np.ascontiguousarray(np.asarray(x, dtype=np.float32))
    w_qkv = np.asarray(w_qkv, dtype=np.float32)
    b_qkv = np.asarray(b_qkv, dtype=np.float32)
    w_out = np.asarray(w_out, dtype=np.float32)
    in_maps = []
    for c in range(N_CORES):
        b = c // 4
        g = c % 4
        r = 256 * g
        wq = w_qkv[r:r + 256]             # [256, 1024]
        wk = w_qkv[1024 + r:1024 + r + 256]
        wv = w_qkv[2048 + r:2048 + r + 256]
        bq = b_qkv[r:r + 256]
        bk = b_qkv[1024 + r:1024 + r + 256]
        bvv = b_qkv[2048 + r:2048 + r + 256]
        in_maps.append({
            "xT": np.ascontiguousarray(x[b].T).astype(NP_DT),
            "wqkT": np.ascontiguousarray(np.concatenate([wq, wk], 0).T).astype(NP_DT),
            "wvT": np.ascontiguousarray(wv.T).astype(NP_DT),
            "woT": np.ascontiguousarray(w_out[:, r:r + 256].T).astype(NP_DT),
            "bqkT": np.ascontiguousarray(
                np.stack([bq[:128], bq[128:], bk[:128], bk[128:]], axis=1)
            ),
            "bv": np.ascontiguousarray(bvv[None, :]).astype(NP_DT),
            "onesA": np.ones((1, 128), dtype=NP_DT),
            "onesAr": np.ones((1, 64), dtype=np.float32),
            "onesB": np.ones((128, 4), dtype=NP_DT),
        })
    return in_maps


def combine(results, b_out):
    """Sum per-core partials within each batch and add output bias."""
    b_out = np.asarray(b_out, dtype=np.float64)
    outs = []
    for b in range(2):
        acc = np.zeros((S, D), dtype=np.float64)
        for g in range(4):
            acc += results[4 * b + g]["out"].astype(np.float64)
        outs.append(acc + b_out)
    return np.stack(outs).astype(np.float32)


def kernel(x, w_qkv, b_qkv, w_out, b_out):
    if "nc" not in _CACHE:
        _CACHE["nc"] = build()
    nc = _CACHE["nc"]
    in_maps = make_in_maps(x, w_qkv, b_qkv, w_out)
    res = run_bass_kernel_spmd(nc, in_maps, list(range(N_CORES)))
    return combine(res.results, b_out)
